# revision 11
# baseline (speedup 1.0000x reference)
"""CategorySpecificLinear TRN2 kernel.

out[b] = x[b] @ W[cat_ids[b]] + bias[cat_ids[b]]
  x: [64, 512, 1024] f32, W: [32, 1024, 4096] f32, b: [32, 4096] f32
  -> out [64, 512, 4096] f32

Strategy: data-parallel over batch — 8 batches per core on 8 NeuronCores.
The category gather, fp16 conversion, and x transpose are done on the host
(cat_ids are known at launch), so each core receives its 8 per-batch weight
matrices directly; no on-device indexing is needed. Matmuls run in fp16 with
fp32 PSUM accumulation: same PE throughput as bf16 on TRN2 (1 cycle/row) but
~8x better accuracy (~4e-4 rel), and 4x faster than native fp32 (4 cycles/row).

Per core: 2048 matmuls of [128k,128m]@[128k,512n]; the trace shows the steady
state already at the 216 ns warm issue-rate floor, so the design minimizes
edge losses and DMA-latency fragility:
  - W for a whole batch lives in one SBUF tile (64 KB/partition, 2 batches
    buffered). Batches 1+ load it with 8 fat k-slab DMAs (8 KB contiguous
    descriptors) issued a full batch (~55 us of slack) ahead, so a multi-us
    HBM latency spike cannot stall the PE — the per-n-tile streaming flow
    control that caused batch-boundary stalls is gone entirely.
  - batch 0 is instead loaded in consumption order (n-major halves, k-pieces
    for n0) so the first matmul only waits on ~256 KB.
  - inputs ride two parallel HWDGE queues (sync + gpsimd, W slabs alternate;
    x on gpsimd) and outputs a third (scalar): no stream blocks another.
  - output is written as fp16 (upcast on host): halves out traffic, cutting
    total HBM load per core from 142 MB to 108 MB (DMA was 94% active).
  - the bias add happens on the host in fp32 (b[cat_ids] broadcast over seq).
    This keeps gpsimd free of compute ops (its partition_broadcast cost 6 us
    per batch and once stalled the whole psum-drain chain), and the device
    epilogue is a bare psum->sbuf fp16 copy on the vector engine.
"""
import numpy as np

B_TOTAL = 64
N_CORES = 8
B = B_TOTAL // N_CORES  # batches per core
S = 512    # seq
K = 1024   # input_dim
H = 4096   # hidden_dim
P = 128
KT = K // P   # 8 k-tiles
MT = S // P   # 4 m-tiles
NW = 512      # hidden tile width (one fp32 PSUM bank)
NT = H // NW  # 8 n-tiles

_NC = None


def _build_nc():
    global _NC
    if _NC is not None:
        return _NC

    import concourse.mybir as mybir
    import concourse.tile as tile
    from concourse import bacc

    f16 = mybir.dt.float16
    f32 = mybir.dt.float32

    nc = bacc.Bacc("TRN2", target_bir_lowering=False, debug=False, num_devices=N_CORES)
    xt = nc.dram_tensor("xt", [B, K, S], f16, kind="ExternalInput").ap()
    w = nc.dram_tensor("w", [B, K, H], f16, kind="ExternalInput").ap()
    out = nc.dram_tensor("out", [B, S, H], f16, kind="ExternalOutput").ap()

    with tile.TileContext(nc) as tc:
        with (
            tc.tile_pool(name="xtp", bufs=3) as xtp,
            tc.tile_pool(name="wp", bufs=2) as wp,
            tc.tile_pool(name="op", bufs=8) as op,
            tc.tile_pool(name="ps", bufs=8, space="PSUM") as ps,
        ):
            for b_i in range(B):
                xt_sb = xtp.tile([P, KT, S], f16, tag="xt")
                if b_i == 0:
                    # k-granular so the first matmul only waits on 128 KB
                    for k_i in range(KT):
                        nc.gpsimd.dma_start(
                            xt_sb[:, k_i, :], xt[b_i, k_i * P : (k_i + 1) * P, :]
                        )
                else:
                    for sp in range(2):
                        k0, k1 = sp * (KT // 2), (sp + 1) * (KT // 2)
                        nc.gpsimd.dma_start(
                            xt_sb[:, k0:k1, :],
                            xt[b_i, k0 * P : k1 * P, :].rearrange(
                                "(ko p) s -> p ko s", p=P
                            ),
                        )
                w_sb = wp.tile([P, KT, H], f16, tag="w")
                if b_i == 0:
                    # consumption order: n-major, and k-granular for n0 so
                    # the pipeline fills with minimal first-tile latency;
                    # halves alternate between the sync and gpsimd queues
                    for k_i in range(KT):
                        nc.sync.dma_start(
                            w_sb[:, k_i, 0:NW],
                            w[b_i, k_i * P : (k_i + 1) * P, 0:NW],
                        )
                    for n_i in range(1, NT):
                        eng = nc.sync if n_i % 2 == 0 else nc.gpsimd
                        for sp in range(2):
                            k0, k1 = sp * (KT // 2), (sp + 1) * (KT // 2)
                            eng.dma_start(
                                w_sb[:, k0:k1, n_i * NW : (n_i + 1) * NW],
                                w[b_i, k0 * P : k1 * P, n_i * NW : (n_i + 1) * NW].rearrange(
                                    "(ko p) n -> p ko n", p=P
                                ),
                            )
                else:
                    # fat k-slab DMAs: 128 x 8 KB fully-contiguous descriptor
                    # rows each, issued a whole batch ahead of consumption
                    for k_i in range(KT):
                        eng = nc.sync if k_i % 2 == 0 else nc.gpsimd
                        eng.dma_start(
                            w_sb[:, k_i, :],
                            w[b_i, k_i * P : (k_i + 1) * P, :],
                        )
                for n_i in range(NT):
                    for m_i in range(MT):
                        pt = ps.tile([P, NW], f32, tag="psum")
                        for k_i in range(KT):
                            nc.tensor.matmul(
                                pt[:],
                                xt_sb[:, k_i, m_i * P : (m_i + 1) * P],
                                w_sb[:, k_i, n_i * NW : (n_i + 1) * NW],
                                start=(k_i == 0),
                                stop=(k_i == KT - 1),
                            )
                        ot = op.tile([P, NW], f16, tag="out")
                        nc.vector.tensor_scalar_add(ot[:], pt[:], 0.0)
                        # outputs get their own HWDGE queue (scalar) so the
                        # bursts can't head-of-line-block the input streams
                        nc.scalar.dma_start(
                            out[b_i, m_i * P : (m_i + 1) * P, n_i * NW : (n_i + 1) * NW],
                            ot[:],
                        )
    nc.compile()
    _NC = nc
    return nc


def _prep_in_maps(x, cat_ids, W, b):
    W16 = W.astype(np.float16)                      # [32, K, H]
    Wg = W16[cat_ids]                               # [64, K, H]
    x16 = x.astype(np.float16)                      # [64, S, K]
    xt16 = np.ascontiguousarray(x16.transpose(0, 2, 1))  # [64, K, S]

    in_maps = []
    for c in range(N_CORES):
        sl = slice(B * c, B * (c + 1))
        in_maps.append(
            {
                "xt": np.ascontiguousarray(xt16[sl]),
                "w": np.ascontiguousarray(Wg[sl]),
            }
        )
    return in_maps


def kernel(x, cat_ids, W, b):
    from concourse.bass_utils import run_bass_kernel_spmd

    x = np.asarray(x, dtype=np.float32)
    cat_ids = np.asarray(cat_ids).astype(np.int64)
    W = np.asarray(W, dtype=np.float32)
    b = np.asarray(b, dtype=np.float32)

    nc = _build_nc()
    in_maps = _prep_in_maps(x, cat_ids, W, b)
    res = run_bass_kernel_spmd(nc, in_maps, core_ids=list(range(N_CORES)))
    out = np.concatenate([r["out"] for r in res.results], axis=0)
    # bias add on host in fp32 (device returns the bare fp16 matmul result)
    out = out.astype(np.float32)
    out += b[cat_ids][:, None, :]
    return out


# revision 12
# speedup vs baseline: 1.0279x; 1.0279x over previous
"""CategorySpecificLinear TRN2 kernel.

out[b] = x[b] @ W[cat_ids[b]] + bias[cat_ids[b]]
  x: [64, 512, 1024] f32, W: [32, 1024, 4096] f32, b: [32, 4096] f32
  -> out [64, 512, 4096] f32

Strategy: data-parallel over batch — 8 batches per core on 8 NeuronCores.
The category gather, fp16 conversion, and x transpose are done on the host
(cat_ids are known at launch), so each core receives its 8 per-batch weight
matrices directly; no on-device indexing is needed. Matmuls run in fp16 with
fp32 PSUM accumulation: same PE throughput as bf16 on TRN2 (1 cycle/row) but
~8x better accuracy (~4e-4 rel), and 4x faster than native fp32 (4 cycles/row).

Per core: 2048 matmuls of [128k,128m]@[128k,512n]; the trace shows the steady
state already at the 216 ns warm issue-rate floor, so the remaining wins are
at the edges:
  - w loads ride the sync HWDGE queue alone with a deep (12-tile) ring, so a
    multi-us HBM latency spike never reaches the PE; x rides the gpsimd queue
    so the first x tile and the w stream transfer in parallel and the
    per-batch x prefetch is never head-of-line-blocked behind 16 queued w
    DMAs; outputs get a third queue (scalar).
  - batch 0 is loaded k-granularly (128 KB pieces) so the first matmul's
    dependency is ~256 KB, not ~1.3 MB: first matmul starts ~6 us earlier.
  - output is written as fp16 (upcast on host): halves out traffic, cutting
    total HBM load per core from 142 MB to 108 MB (DMA was 94% active).
  - the bias add happens on the host in fp32 (b[cat_ids] broadcast over seq).
    This keeps gpsimd free of compute ops (its partition_broadcast cost 6 us
    per batch and once stalled the whole psum-drain chain), and the device
    epilogue is a bare psum->sbuf fp16 copy on the vector engine.
"""
import numpy as np

B_TOTAL = 64
N_CORES = 8
B = B_TOTAL // N_CORES  # batches per core
S = 512    # seq
K = 1024   # input_dim
H = 4096   # hidden_dim
P = 128
KT = K // P   # 8 k-tiles
MT = S // P   # 4 m-tiles
NW = 512      # hidden tile width (one fp32 PSUM bank)
NT = H // NW  # 8 n-tiles

_NC = None


def _build_nc():
    global _NC
    if _NC is not None:
        return _NC

    import concourse.mybir as mybir
    import concourse.tile as tile
    from concourse import bacc

    f16 = mybir.dt.float16
    f32 = mybir.dt.float32

    nc = bacc.Bacc("TRN2", target_bir_lowering=False, debug=False, num_devices=N_CORES)
    xt = nc.dram_tensor("xt", [B, K, S], f16, kind="ExternalInput").ap()
    w = nc.dram_tensor("w", [B, K, H], f16, kind="ExternalInput").ap()
    out = nc.dram_tensor("out", [B, S, H], f16, kind="ExternalOutput").ap()

    with tile.TileContext(nc) as tc:
        with (
            tc.tile_pool(name="xtp", bufs=2) as xtp,
            tc.tile_pool(name="wp", bufs=12) as wp,
            tc.tile_pool(name="op", bufs=8) as op,
            tc.tile_pool(name="ps", bufs=8, space="PSUM") as ps,
        ):
            for b_i in range(B):
                xt_sb = xtp.tile([P, KT, S], f16, tag="xt")
                if b_i == 0:
                    # k-granular so the first matmul only waits on 128 KB
                    for k_i in range(KT):
                        nc.gpsimd.dma_start(
                            xt_sb[:, k_i, :], xt[b_i, k_i * P : (k_i + 1) * P, :]
                        )
                else:
                    for sp in range(2):
                        k0, k1 = sp * (KT // 2), (sp + 1) * (KT // 2)
                        nc.gpsimd.dma_start(
                            xt_sb[:, k0:k1, :],
                            xt[b_i, k0 * P : k1 * P, :].rearrange(
                                "(ko p) s -> p ko s", p=P
                            ),
                        )
                for n_i in range(NT):
                    w_sb = wp.tile([P, KT, NW], f16, tag="w")
                    if b_i == 0 and n_i == 0:
                        # k-granular in consumption order for the ramp-up
                        for k_i in range(KT):
                            nc.sync.dma_start(
                                w_sb[:, k_i, :],
                                w[b_i, k_i * P : (k_i + 1) * P, 0:NW],
                            )
                    else:
                        for sp in range(2):
                            k0, k1 = sp * (KT // 2), (sp + 1) * (KT // 2)
                            nc.sync.dma_start(
                                w_sb[:, k0:k1, :],
                                w[b_i, k0 * P : k1 * P, n_i * NW : (n_i + 1) * NW].rearrange(
                                    "(ko p) n -> p ko n", p=P
                                ),
                            )
                    for m_i in range(MT):
                        pt = ps.tile([P, NW], f32, tag="psum")
                        for k_i in range(KT):
                            nc.tensor.matmul(
                                pt[:],
                                xt_sb[:, k_i, m_i * P : (m_i + 1) * P],
                                w_sb[:, k_i, :],
                                start=(k_i == 0),
                                stop=(k_i == KT - 1),
                            )
                        ot = op.tile([P, NW], f16, tag="out")
                        nc.vector.tensor_scalar_add(ot[:], pt[:], 0.0)
                        # outputs get their own HWDGE queue (scalar) so the
                        # bursts can't head-of-line-block the input streams
                        nc.scalar.dma_start(
                            out[b_i, m_i * P : (m_i + 1) * P, n_i * NW : (n_i + 1) * NW],
                            ot[:],
                        )
    nc.compile()
    _NC = nc
    return nc


def _prep_in_maps(x, cat_ids, W, b):
    W16 = W.astype(np.float16)                      # [32, K, H]
    Wg = W16[cat_ids]                               # [64, K, H]
    x16 = x.astype(np.float16)                      # [64, S, K]
    xt16 = np.ascontiguousarray(x16.transpose(0, 2, 1))  # [64, K, S]

    in_maps = []
    for c in range(N_CORES):
        sl = slice(B * c, B * (c + 1))
        in_maps.append(
            {
                "xt": np.ascontiguousarray(xt16[sl]),
                "w": np.ascontiguousarray(Wg[sl]),
            }
        )
    return in_maps


def kernel(x, cat_ids, W, b):
    from concourse.bass_utils import run_bass_kernel_spmd

    x = np.asarray(x, dtype=np.float32)
    cat_ids = np.asarray(cat_ids).astype(np.int64)
    W = np.asarray(W, dtype=np.float32)
    b = np.asarray(b, dtype=np.float32)

    nc = _build_nc()
    in_maps = _prep_in_maps(x, cat_ids, W, b)
    res = run_bass_kernel_spmd(nc, in_maps, core_ids=list(range(N_CORES)))
    out = np.concatenate([r["out"] for r in res.results], axis=0)
    # bias add on host in fp32 (device returns the bare fp16 matmul result)
    out = out.astype(np.float32)
    out += b[cat_ids][:, None, :]
    return out


# revision 13
# speedup vs baseline: 1.0469x; 1.0185x over previous
"""CategorySpecificLinear TRN2 kernel.

out[b] = x[b] @ W[cat_ids[b]] + bias[cat_ids[b]]
  x: [64, 512, 1024] f32, W: [32, 1024, 4096] f32, b: [32, 4096] f32
  -> out [64, 512, 4096] f32

Strategy: data-parallel over batch — 8 batches per core on 8 NeuronCores.
The category gather, fp16 conversion, and x transpose are done on the host
(cat_ids are known at launch), so each core receives its 8 per-batch weight
matrices directly; no on-device indexing is needed. Matmuls run in fp16 with
fp32 PSUM accumulation: same PE throughput as bf16 on TRN2 (1 cycle/row) but
~8x better accuracy (~4e-4 rel), and 4x faster than native fp32 (4 cycles/row).

Per core: 2048 matmuls of [128k,128m]@[128k,512n]; the trace shows the steady
state already at the 216 ns warm issue-rate floor, so the remaining wins are
at the edges:
  - w loads ride the sync HWDGE queue alone with a deep (8-tile) ring, so a
    multi-us HBM latency spike never reaches the PE; x rides the gpsimd queue
    so the first x tile and the w stream transfer in parallel and the
    per-batch x prefetch is never head-of-line-blocked behind 16 queued w
    DMAs; outputs get a third queue (scalar).
  - batch 0 is loaded k-granularly (128 KB pieces) so the first matmul's
    dependency is ~256 KB, not ~1.3 MB: first matmul starts ~6 us earlier.
  - output is written as fp16 (upcast on host): halves out traffic, cutting
    total HBM load per core from 142 MB to 108 MB (DMA was 94% active).
  - the bias add happens on the host in fp32 (b[cat_ids] broadcast over seq).
    This keeps gpsimd free of compute ops (its partition_broadcast cost 6 us
    per batch and once stalled the whole psum-drain chain), and the device
    epilogue is a bare psum->sbuf fp16 copy on the vector engine.
"""
import numpy as np

B_TOTAL = 64
N_CORES = 8
B = B_TOTAL // N_CORES  # batches per core
S = 512    # seq
K = 1024   # input_dim
H = 4096   # hidden_dim
P = 128
KT = K // P   # 8 k-tiles
MT = S // P   # 4 m-tiles
NW = 512      # hidden tile width (one fp32 PSUM bank)
NT = H // NW  # 8 n-tiles

_NC = None


def _build_nc():
    global _NC
    if _NC is not None:
        return _NC

    import concourse.mybir as mybir
    import concourse.tile as tile
    from concourse import bacc

    f16 = mybir.dt.float16
    f32 = mybir.dt.float32

    nc = bacc.Bacc("TRN2", target_bir_lowering=False, debug=False, num_devices=N_CORES)
    xt = nc.dram_tensor("xt", [B, K, S], f16, kind="ExternalInput").ap()
    w = nc.dram_tensor("w", [B, K, H], f16, kind="ExternalInput").ap()
    out = nc.dram_tensor("out", [B, S, H], f16, kind="ExternalOutput").ap()

    with tile.TileContext(nc) as tc:
        with (
            tc.tile_pool(name="xtp", bufs=2) as xtp,
            tc.tile_pool(name="wp", bufs=8) as wp,
            tc.tile_pool(name="op", bufs=8) as op,
            tc.tile_pool(name="ps", bufs=8, space="PSUM") as ps,
        ):
            for b_i in range(B):
                xt_sb = xtp.tile([P, KT, S], f16, tag="xt")
                if b_i == 0:
                    # k-granular so the first matmul only waits on 128 KB
                    for k_i in range(KT):
                        nc.gpsimd.dma_start(
                            xt_sb[:, k_i, :], xt[b_i, k_i * P : (k_i + 1) * P, :]
                        )
                else:
                    for sp in range(2):
                        k0, k1 = sp * (KT // 2), (sp + 1) * (KT // 2)
                        nc.gpsimd.dma_start(
                            xt_sb[:, k0:k1, :],
                            xt[b_i, k0 * P : k1 * P, :].rearrange(
                                "(ko p) s -> p ko s", p=P
                            ),
                        )
                for n_i in range(NT):
                    w_sb = wp.tile([P, KT, NW], f16, tag="w")
                    if b_i == 0 and n_i == 0:
                        # k-granular in consumption order for the ramp-up
                        for k_i in range(KT):
                            nc.sync.dma_start(
                                w_sb[:, k_i, :],
                                w[b_i, k_i * P : (k_i + 1) * P, 0:NW],
                            )
                    else:
                        for sp in range(2):
                            k0, k1 = sp * (KT // 2), (sp + 1) * (KT // 2)
                            nc.sync.dma_start(
                                w_sb[:, k0:k1, :],
                                w[b_i, k0 * P : k1 * P, n_i * NW : (n_i + 1) * NW].rearrange(
                                    "(ko p) n -> p ko n", p=P
                                ),
                            )
                    for m_i in range(MT):
                        pt = ps.tile([P, NW], f32, tag="psum")
                        for k_i in range(KT):
                            nc.tensor.matmul(
                                pt[:],
                                xt_sb[:, k_i, m_i * P : (m_i + 1) * P],
                                w_sb[:, k_i, :],
                                start=(k_i == 0),
                                stop=(k_i == KT - 1),
                            )
                        ot = op.tile([P, NW], f16, tag="out")
                        nc.vector.tensor_scalar_add(ot[:], pt[:], 0.0)
                        # outputs get their own HWDGE queue (scalar) so the
                        # bursts can't head-of-line-block the input streams
                        nc.scalar.dma_start(
                            out[b_i, m_i * P : (m_i + 1) * P, n_i * NW : (n_i + 1) * NW],
                            ot[:],
                        )
    nc.compile()
    _NC = nc
    return nc


def _prep_in_maps(x, cat_ids, W, b):
    W16 = W.astype(np.float16)                      # [32, K, H]
    Wg = W16[cat_ids]                               # [64, K, H]
    x16 = x.astype(np.float16)                      # [64, S, K]
    xt16 = np.ascontiguousarray(x16.transpose(0, 2, 1))  # [64, K, S]

    in_maps = []
    for c in range(N_CORES):
        sl = slice(B * c, B * (c + 1))
        in_maps.append(
            {
                "xt": np.ascontiguousarray(xt16[sl]),
                "w": np.ascontiguousarray(Wg[sl]),
            }
        )
    return in_maps


def kernel(x, cat_ids, W, b):
    from concourse.bass_utils import run_bass_kernel_spmd

    x = np.asarray(x, dtype=np.float32)
    cat_ids = np.asarray(cat_ids).astype(np.int64)
    W = np.asarray(W, dtype=np.float32)
    b = np.asarray(b, dtype=np.float32)

    nc = _build_nc()
    in_maps = _prep_in_maps(x, cat_ids, W, b)
    res = run_bass_kernel_spmd(nc, in_maps, core_ids=list(range(N_CORES)))
    out = np.concatenate([r["out"] for r in res.results], axis=0)
    # bias add on host in fp32 (device returns the bare fp16 matmul result)
    out = out.astype(np.float32)
    out += b[cat_ids][:, None, :]
    return out
